# revision 11
# baseline (speedup 1.0000x reference)
"""Bass/Trainium2 kernel for KnowledgeConsistentAttention (first-call forward).

Reference math (per image):
    kern = normalize(fg.reshape(C, H*W).T + eps)          # [P, C], P = H*W
    scores = kern @ fg.reshape(C, H*W)                    # [P, YX]
    scores = sum_pool3x3(scores over (y, x))
    att = softmax(scores, axis=P)
    out = kern.T @ att                                    # [C, YX]

Key identities used:
  * The 3x3 zero-padded sum pool acts on the RHS spatial axes only, so
    pool(kern @ fg) == kern @ pool(fg): pool the (tiny) input once instead
    of the (huge) scores.
  * softmax then kern.T @ att == (kern.T @ exp(s)) / (ones @ exp(s)):
    append a ones-column to kern so one matmul produces both numerator and
    denominator; divide at the end.  Scores are in [-30, 30] for this
    distribution, so exp() cannot overflow fp32 and no max-subtraction is
    needed.

Sharding: data-parallel, 8 cores = 4 images x 2 y-halves.  Each core:
  GEMM1 (fp16) scores = kern_t.T @ fg2, two p-tiles packed into row-group
               halves of the PE array (K=64 each) -> concurrent.
  ACT          e = exp(scores) -> SBUF (bf16)
  GEMM2 (bf16) kern split into 4 quarters of 16 channels (+ ones column
               each) issued to 4 PE column groups (tile_position) ->
               concurrent; accumulate 32 p-tiles in PSUM.
Host does the cheap prep (normalize, pool, layouts) and the final divide.
"""

import os
import numpy as np

B, C, H, W = 4, 64, 64, 64
P = H * W            # 4096 dynamic kernels (one per pixel)
YXH = (H // 2) * W   # 2048 output columns per core (half image)
EPS = 1e-7

NP_TILES = P // 128  # 32 p-tiles
NPAIRS = NP_TILES // 2
CHUNK = 512          # yx columns per psum bank
NCHUNK = YXH // CHUNK
OUTROWS = 68         # logical out rows; device tensor uses 128 (32-aligned quarters)

_CACHE = {}
G1DT = "float16"    # GEMM1 operand dtype (kt, rhs)
G2DT = "bfloat16"   # GEMM2 operand dtype (ka, e)
TRACE = False
LAST_RESULTS = None


def _build_program():
    import concourse.bacc as bacc
    import concourse.mybir as mybir
    import concourse.tile as tile
    from contextlib import ExitStack

    f32 = mybir.dt.float32
    g1dt = getattr(mybir.dt, G1DT)
    g2dt = getattr(mybir.dt, G2DT)

    nc = bacc.Bacc("TRN2", target_bir_lowering=False, debug=False, num_devices=8)
    # kt2: pair layout — rows 0:64 even p-tiles, rows 64:128 odd p-tiles
    kt_d = nc.dram_tensor("kt2", [128, NPAIRS * 128], g1dt, kind="ExternalInput").ap()
    # ka2: per p-tile 4 column-group quarters of 17 cols (16 kern + ones)
    ka_d = nc.dram_tensor("ka2", [128, NP_TILES * OUTROWS], g2dt, kind="ExternalInput").ap()
    # rhs2: pooled fg half, duplicated into both row-group halves
    rhs_d = nc.dram_tensor("rhs2", [128, YXH], g1dt, kind="ExternalInput").ap()
    out_d = nc.dram_tensor("out_aug", [128, YXH], f32, kind="ExternalOutput").ap()

    with tile.TileContext(nc) as tc, ExitStack() as ctx:
        const = ctx.enter_context(tc.tile_pool(name="const", bufs=1))
        # Split input DMAs so the first matmuls only wait on their slices.
        rhs = const.tile([128, YXH], g1dt)
        for ci in range(NCHUNK):
            cc = slice(ci * CHUNK, (ci + 1) * CHUNK)
            nc.sync.dma_start(rhs[:, cc], rhs_d[:, cc])
        kt = const.tile([128, NPAIRS * 128], g1dt)
        for qi in range(4):
            qc = slice(qi * 4 * 128, (qi + 1) * 4 * 128)
            nc.sync.dma_start(kt[:, qc], kt_d[:, qc])
        ka = const.tile([128, NP_TILES * OUTROWS], g2dt)
        for hi in range(2):
            hc = slice(hi * 16 * OUTROWS, (hi + 1) * 16 * OUTROWS)
            nc.sync.dma_start(ka[:, hc], ka_d[:, hc])

        spool = ctx.enter_context(tc.tile_pool(name="spool", bufs=2, space="PSUM"))
        opool = ctx.enter_context(tc.tile_pool(name="opool", bufs=2, space="PSUM"))
        epool = ctx.enter_context(tc.tile_pool(name="epool", bufs=3))
        obuf = ctx.enter_context(tc.tile_pool(name="obuf", bufs=2))

        stages = [(ci, pi) for ci in range(NCHUNK) for pi in range(NPAIRS)]
        s_tiles = [None] * len(stages)

        def emit_gemm1(k):
            ci, pi = stages[k]
            s = spool.tile([128, 2 * CHUNK], f32, tag="s")
            s_tiles[k] = s
            pcols = slice(pi * 128, (pi + 1) * 128)
            ccols = slice(ci * CHUNK, (ci + 1) * CHUNK)
            nc.tensor.matmul(s[:, 0:CHUNK], kt[0:64, pcols], rhs[0:64, ccols],
                             start=True, stop=True, tile_position=(0, 0))
            nc.tensor.matmul(s[:, CHUNK:2 * CHUNK], kt[64:128, pcols],
                             rhs[64:128, ccols],
                             start=True, stop=True, tile_position=(64, 0))

        osum = None
        emit_gemm1(0)
        for k, (ci, pi) in enumerate(stages):
            if k + 1 < len(stages):
                emit_gemm1(k + 1)
            if pi == 0:
                osum = opool.tile([128, CHUNK], f32, tag="osum")
            s = s_tiles[k]
            e = epool.tile([128, 2 * CHUNK], g2dt, tag="e")
            nc.scalar.activation(e[:], s[:], mybir.ActivationFunctionType.Exp)
            # p-tile 2*pi (kt2 rows 0:64 -> e slot 0), 2*pi+1 (slot 1)
            for j in range(2):
                t = 2 * pi + j
                for q in range(4):
                    nc.tensor.matmul(
                        osum[32 * q:32 * q + 17, :],
                        ka[:, t * OUTROWS + 17 * q: t * OUTROWS + 17 * (q + 1)],
                        e[:, j * CHUNK:(j + 1) * CHUNK],
                        start=(t == 0), stop=(t == NP_TILES - 1),
                        tile_position=(0, 32 * q),
                        skip_group_check=True,
                    )
            s_tiles[k] = None
            if pi == NPAIRS - 1:
                ob = obuf.tile([128, CHUNK], f32, tag="ob")
                for q in range(4):
                    nc.vector.tensor_copy(ob[32 * q:32 * q + 17, :],
                                          osum[32 * q:32 * q + 17, :])
                nc.sync.dma_start(out_d[:, ci * CHUNK:(ci + 1) * CHUNK], ob[:])
    nc.compile()
    return nc


def _get_program():
    if "nc" not in _CACHE:
        _CACHE["nc"] = _build_program()
    return _CACHE["nc"]


def _pool3x3(x):
    # 3x3 stride-1 zero-padded sum pool over the last two axes.
    p = np.pad(x, ((0, 0), (0, 0), (1, 1), (0, 0)))
    x = p[:, :, :-2] + p[:, :, 1:-1] + p[:, :, 2:]
    p = np.pad(x, ((0, 0), (0, 0), (0, 0), (1, 1)))
    return p[:, :, :, :-2] + p[:, :, :, 1:-1] + p[:, :, :, 2:]


def kernel(foreground, masks=None, **_unused):
    global LAST_RESULTS
    from concourse import bass_utils
    import ml_dtypes

    _np_dt = {"bfloat16": ml_dtypes.bfloat16, "float16": np.float16,
              "float32r": np.float32}
    g1np, g2np = _np_dt[G1DT], _np_dt[G2DT]

    fg = np.ascontiguousarray(np.asarray(foreground, dtype=np.float32))
    assert fg.shape == (B, C, H, W)

    # kern_t[c, p] = normalized (fg + eps), kern transposed
    kt_all = fg.reshape(B, C, P) + EPS
    kt_all = kt_all / np.sqrt(
        (kt_all.astype(np.float64) ** 2).sum(1, keepdims=True)).astype(np.float32)
    # kt2: [128, NPAIRS*128] — even p-tiles in rows 0:64, odd in rows 64:128
    kt_r = kt_all.reshape(B, C, NPAIRS, 2, 128)
    kt2 = np.concatenate([kt_r[:, :, :, 0, :].reshape(B, C, NPAIRS * 128),
                          kt_r[:, :, :, 1, :].reshape(B, C, NPAIRS * 128)],
                         axis=1).astype(g1np)
    # ka2: [128, NP_TILES*68] — per p-tile, 4 quarters of (16 kern cols + ones)
    kq = kt_all.transpose(0, 2, 1).reshape(B, NP_TILES, 128, 4, 16)
    kq = np.concatenate([kq, np.ones((B, NP_TILES, 128, 4, 1), np.float32)], -1)
    ka2 = np.ascontiguousarray(kq.transpose(0, 2, 1, 3, 4)).reshape(
        B, 128, NP_TILES * OUTROWS).astype(g2np)

    fg2 = _pool3x3(fg)

    in_maps = []
    for core in range(8):
        b, yh = core // 2, core % 2
        half = fg2[b, :, yh * (H // 2):(yh + 1) * (H // 2), :].reshape(C, YXH)
        in_maps.append({
            "kt2": np.ascontiguousarray(kt2[b]),
            "ka2": np.ascontiguousarray(ka2[b]),
            "rhs2": np.concatenate([half, half], axis=0).astype(g1np),
        })

    nc = _get_program()
    res = bass_utils.run_bass_kernel_spmd(
        nc, in_maps, core_ids=list(range(8)), trace=TRACE)
    LAST_RESULTS = res

    out = np.empty((B, C, H, W), dtype=np.float32)
    for core in range(8):
        b, yh = core // 2, core % 2
        oa = res.results[core]["out_aug"]  # [128, YXH]
        den = oa[16]                       # ones-row of quarter 0
        num = np.concatenate([oa[32 * q:32 * q + 16] for q in range(4)], axis=0)
        img = num / den
        out[b, :, yh * (H // 2):(yh + 1) * (H // 2), :] = img.reshape(C, H // 2, W)
    return out


# revision 12
# speedup vs baseline: 1.0434x; 1.0434x over previous
"""Bass/Trainium2 kernel for KnowledgeConsistentAttention (first-call forward).

Reference math (per image):
    kern = normalize(fg.reshape(C, H*W).T + eps)          # [P, C], P = H*W
    scores = kern @ fg.reshape(C, H*W)                    # [P, YX]
    scores = sum_pool3x3(scores over (y, x))
    att = softmax(scores, axis=P)
    out = kern.T @ att                                    # [C, YX]

Key identities used:
  * The 3x3 zero-padded sum pool acts on the RHS spatial axes only, so
    pool(kern @ fg) == kern @ pool(fg): pool the (tiny) input once instead
    of the (huge) scores.
  * softmax then kern.T @ att == (kern.T @ exp(s)) / (ones @ exp(s)):
    append a ones-column to kern so one matmul produces both numerator and
    denominator; divide at the end.  Scores are in [-30, 30] for this
    distribution, so exp() cannot overflow fp32 and no max-subtraction is
    needed.

Sharding: data-parallel, 8 cores = 4 images x 2 y-halves.  Each core:
  GEMM1 (fp16) scores = kern_t.T @ fg2, two p-tiles packed into row-group
               halves of the PE array (K=64 each) -> concurrent.
  ACT          e = exp(scores) -> SBUF (bf16)
  GEMM2 (bf16) kern split into 4 quarters of 16 channels (+ ones column
               each) issued to 4 PE column groups (tile_position) ->
               concurrent; accumulate 32 p-tiles in PSUM.
Host does the cheap prep (normalize, pool, layouts) and the final divide.
"""

import os
import numpy as np

B, C, H, W = 4, 64, 64, 64
P = H * W            # 4096 dynamic kernels (one per pixel)
YXH = (H // 2) * W   # 2048 output columns per core (half image)
EPS = 1e-7

NP_TILES = P // 128  # 32 p-tiles
NPAIRS = NP_TILES // 2
CHUNK = 512          # yx columns per psum bank
NCHUNK = YXH // CHUNK
OUTROWS = 68         # logical out rows; device tensor uses 128 (32-aligned quarters)

_CACHE = {}
G1DT = "float16"    # GEMM1 operand dtype (kt, rhs)
G2DT = "bfloat16"   # GEMM2 operand dtype (ka, e)
TRACE = False
LAST_RESULTS = None


def _build_program():
    import concourse.bacc as bacc
    import concourse.mybir as mybir
    import concourse.tile as tile
    from contextlib import ExitStack

    f32 = mybir.dt.float32
    g1dt = getattr(mybir.dt, G1DT)
    g2dt = getattr(mybir.dt, G2DT)

    nc = bacc.Bacc("TRN2", target_bir_lowering=False, debug=False, num_devices=8)
    # kt2: pair layout — rows 0:64 even p-tiles, rows 64:128 odd p-tiles
    kt_d = nc.dram_tensor("kt2", [128, NPAIRS * 128], g1dt, kind="ExternalInput").ap()
    # ka2: per p-tile 4 column-group quarters of 17 cols (16 kern + ones)
    ka_d = nc.dram_tensor("ka2", [128, NP_TILES * OUTROWS], g2dt, kind="ExternalInput").ap()
    # rhs2: pooled fg half, duplicated into both row-group halves
    rhs_d = nc.dram_tensor("rhs2", [128, YXH], g1dt, kind="ExternalInput").ap()
    out_d = nc.dram_tensor("out_aug", [128, YXH], f32, kind="ExternalOutput").ap()

    with tile.TileContext(nc) as tc, ExitStack() as ctx:
        const = ctx.enter_context(tc.tile_pool(name="const", bufs=1))
        # Split input DMAs across queues so the first matmuls only wait on
        # their own slices (sync + scalar are HWDGE queues, gpsimd SWDGE).
        kt = const.tile([128, NPAIRS * 128], g1dt)
        rhs = const.tile([128, YXH], g1dt)
        for qi in range(4):
            qc = slice(qi * 4 * 128, (qi + 1) * 4 * 128)
            nc.sync.dma_start(kt[:, qc], kt_d[:, qc])
        for ci in range(NCHUNK):
            cc = slice(ci * CHUNK, (ci + 1) * CHUNK)
            nc.scalar.dma_start(rhs[:, cc], rhs_d[:, cc])
        ka = const.tile([128, NP_TILES * OUTROWS], g2dt)
        for hi in range(4):
            hc = slice(hi * 8 * OUTROWS, (hi + 1) * 8 * OUTROWS)
            nc.gpsimd.dma_start(ka[:, hc], ka_d[:, hc])

        spool = ctx.enter_context(tc.tile_pool(name="spool", bufs=3, space="PSUM"))
        opool = ctx.enter_context(tc.tile_pool(name="opool", bufs=2, space="PSUM"))
        epool = ctx.enter_context(tc.tile_pool(name="epool", bufs=3))
        obuf = ctx.enter_context(tc.tile_pool(name="obuf", bufs=2))

        stages = [(ci, pi) for ci in range(NCHUNK) for pi in range(NPAIRS)]
        s_tiles = [None] * len(stages)

        def emit_gemm1(k):
            ci, pi = stages[k]
            s = spool.tile([128, 2 * CHUNK], f32, tag="s")
            s_tiles[k] = s
            pcols = slice(pi * 128, (pi + 1) * 128)
            ccols = slice(ci * CHUNK, (ci + 1) * CHUNK)
            nc.tensor.matmul(s[:, 0:CHUNK], kt[0:64, pcols], rhs[0:64, ccols],
                             start=True, stop=True, tile_position=(0, 0))
            nc.tensor.matmul(s[:, CHUNK:2 * CHUNK], kt[64:128, pcols],
                             rhs[64:128, ccols],
                             start=True, stop=True, tile_position=(64, 0))

        osum = None
        emit_gemm1(0)
        for k, (ci, pi) in enumerate(stages):
            if k + 1 < len(stages):
                emit_gemm1(k + 1)
            if pi == 0:
                osum = opool.tile([128, CHUNK], f32, tag="osum")
            s = s_tiles[k]
            e = epool.tile([128, 2 * CHUNK], g2dt, tag="e")
            nc.scalar.activation(e[:], s[:], mybir.ActivationFunctionType.Exp)
            # p-tile 2*pi (kt2 rows 0:64 -> e slot 0), 2*pi+1 (slot 1)
            for j in range(2):
                t = 2 * pi + j
                for q in range(4):
                    nc.tensor.matmul(
                        osum[32 * q:32 * q + 17, :],
                        ka[:, t * OUTROWS + 17 * q: t * OUTROWS + 17 * (q + 1)],
                        e[:, j * CHUNK:(j + 1) * CHUNK],
                        start=(t == 0), stop=(t == NP_TILES - 1),
                        tile_position=(0, 32 * q),
                        skip_group_check=True,
                    )
            s_tiles[k] = None
            if pi == NPAIRS - 1:
                ob = obuf.tile([128, CHUNK], f32, tag="ob")
                for q in range(4):
                    nc.vector.tensor_copy(ob[32 * q:32 * q + 17, :],
                                          osum[32 * q:32 * q + 17, :])
                nc.sync.dma_start(out_d[:, ci * CHUNK:(ci + 1) * CHUNK], ob[:])
    nc.compile()
    return nc


def _get_program():
    if "nc" not in _CACHE:
        _CACHE["nc"] = _build_program()
    return _CACHE["nc"]


def _pool3x3(x):
    # 3x3 stride-1 zero-padded sum pool over the last two axes.
    p = np.pad(x, ((0, 0), (0, 0), (1, 1), (0, 0)))
    x = p[:, :, :-2] + p[:, :, 1:-1] + p[:, :, 2:]
    p = np.pad(x, ((0, 0), (0, 0), (0, 0), (1, 1)))
    return p[:, :, :, :-2] + p[:, :, :, 1:-1] + p[:, :, :, 2:]


def kernel(foreground, masks=None, **_unused):
    global LAST_RESULTS
    from concourse import bass_utils
    import ml_dtypes

    _np_dt = {"bfloat16": ml_dtypes.bfloat16, "float16": np.float16,
              "float32r": np.float32}
    g1np, g2np = _np_dt[G1DT], _np_dt[G2DT]

    fg = np.ascontiguousarray(np.asarray(foreground, dtype=np.float32))
    assert fg.shape == (B, C, H, W)

    # kern_t[c, p] = normalized (fg + eps), kern transposed
    kt_all = fg.reshape(B, C, P) + EPS
    kt_all = kt_all / np.sqrt(
        (kt_all.astype(np.float64) ** 2).sum(1, keepdims=True)).astype(np.float32)
    # kt2: [128, NPAIRS*128] — even p-tiles in rows 0:64, odd in rows 64:128
    kt_r = kt_all.reshape(B, C, NPAIRS, 2, 128)
    kt2 = np.concatenate([kt_r[:, :, :, 0, :].reshape(B, C, NPAIRS * 128),
                          kt_r[:, :, :, 1, :].reshape(B, C, NPAIRS * 128)],
                         axis=1).astype(g1np)
    # ka2: [128, NP_TILES*68] — per p-tile, 4 quarters of (16 kern cols + ones)
    kq = kt_all.transpose(0, 2, 1).reshape(B, NP_TILES, 128, 4, 16)
    kq = np.concatenate([kq, np.ones((B, NP_TILES, 128, 4, 1), np.float32)], -1)
    ka2 = np.ascontiguousarray(kq.transpose(0, 2, 1, 3, 4)).reshape(
        B, 128, NP_TILES * OUTROWS).astype(g2np)

    fg2 = _pool3x3(fg)

    in_maps = []
    for core in range(8):
        b, yh = core // 2, core % 2
        half = fg2[b, :, yh * (H // 2):(yh + 1) * (H // 2), :].reshape(C, YXH)
        in_maps.append({
            "kt2": np.ascontiguousarray(kt2[b]),
            "ka2": np.ascontiguousarray(ka2[b]),
            "rhs2": np.concatenate([half, half], axis=0).astype(g1np),
        })

    nc = _get_program()
    res = bass_utils.run_bass_kernel_spmd(
        nc, in_maps, core_ids=list(range(8)), trace=TRACE)
    LAST_RESULTS = res

    out = np.empty((B, C, H, W), dtype=np.float32)
    for core in range(8):
        b, yh = core // 2, core % 2
        oa = res.results[core]["out_aug"]  # [128, YXH]
        den = oa[16]                       # ones-row of quarter 0
        num = np.concatenate([oa[32 * q:32 * q + 16] for q in range(4)], axis=0)
        img = num / den
        out[b, :, yh * (H // 2):(yh + 1) * (H // 2), :] = img.reshape(C, H // 2, W)
    return out


# revision 13
# speedup vs baseline: 1.0628x; 1.0186x over previous
"""Bass/Trainium2 kernel for KnowledgeConsistentAttention (first-call forward).

Reference math (per image):
    kern = normalize(fg.reshape(C, H*W).T + eps)          # [P, C], P = H*W
    scores = kern @ fg.reshape(C, H*W)                    # [P, YX]
    scores = sum_pool3x3(scores over (y, x))
    att = softmax(scores, axis=P)
    out = kern.T @ att                                    # [C, YX]

Key identities used:
  * The 3x3 zero-padded sum pool acts on the RHS spatial axes only, so
    pool(kern @ fg) == kern @ pool(fg): pool the (tiny) input once instead
    of the (huge) scores.
  * softmax then kern.T @ att == (kern.T @ exp(s)) / (ones @ exp(s)):
    append a ones-column to kern so one matmul produces both numerator and
    denominator; divide at the end.  Scores are in [-30, 30] for this
    distribution, so exp() cannot overflow fp32 and no max-subtraction is
    needed.

Sharding: data-parallel, 8 cores = 4 images x 2 y-halves.  Each core:
  GEMM1 (fp16) scores = kern_t.T @ fg2, two p-tiles packed into row-group
               halves of the PE array (K=64 each) -> concurrent.
  ACT          e = exp(scores) -> SBUF (bf16)
  GEMM2 (bf16) kern split into 4 quarters of 16 channels (+ ones column
               each) issued to 4 PE column groups (tile_position) ->
               concurrent; accumulate 32 p-tiles in PSUM.
Host does the cheap prep (normalize, pool, layouts) and the final divide.
"""

import os
import numpy as np

B, C, H, W = 4, 64, 64, 64
P = H * W            # 4096 dynamic kernels (one per pixel)
YXH = (H // 2) * W   # 2048 output columns per core (half image)
EPS = 1e-7

NP_TILES = P // 128  # 32 p-tiles
NPAIRS = NP_TILES // 2
CHUNK = 512          # yx columns per psum bank
NCHUNK = YXH // CHUNK
OUTROWS = 68         # logical out rows; device tensor uses 128 (32-aligned quarters)

_CACHE = {}
G1DT = "float16"    # GEMM1 operand dtype (kt, rhs)
G2DT = "bfloat16"   # GEMM2 operand dtype (ka, e)
TRACE = False
LAST_RESULTS = None


def _build_program():
    import concourse.bacc as bacc
    import concourse.mybir as mybir
    import concourse.tile as tile
    from contextlib import ExitStack

    f32 = mybir.dt.float32
    g1dt = getattr(mybir.dt, G1DT)
    g2dt = getattr(mybir.dt, G2DT)

    nc = bacc.Bacc("TRN2", target_bir_lowering=False, debug=False, num_devices=8)
    # kt2: pair layout — rows 0:64 even p-tiles, rows 64:128 odd p-tiles
    kt_d = nc.dram_tensor("kt2", [128, NPAIRS * 128], g1dt, kind="ExternalInput").ap()
    # ka2: per p-tile 4 column-group quarters of 17 cols (16 kern + ones)
    ka_d = nc.dram_tensor("ka2", [128, NP_TILES * OUTROWS], g2dt, kind="ExternalInput").ap()
    # rhs2: pooled fg half, duplicated into both row-group halves
    rhs_d = nc.dram_tensor("rhs2", [128, YXH], g1dt, kind="ExternalInput").ap()
    out_d = nc.dram_tensor("out_aug", [128, YXH], f32, kind="ExternalOutput").ap()

    with tile.TileContext(nc) as tc, ExitStack() as ctx:
        const = ctx.enter_context(tc.tile_pool(name="const", bufs=1))
        # Split input DMAs across queues so the first matmuls only wait on
        # their own slices (sync + scalar are HWDGE queues, gpsimd SWDGE).
        kt = const.tile([128, NPAIRS * 128], g1dt)
        rhs = const.tile([128, YXH], g1dt)
        for qi in range(4):
            qc = slice(qi * 4 * 128, (qi + 1) * 4 * 128)
            nc.sync.dma_start(kt[:, qc], kt_d[:, qc])
        for ci in range(NCHUNK):
            cc = slice(ci * CHUNK, (ci + 1) * CHUNK)
            nc.scalar.dma_start(rhs[:, cc], rhs_d[:, cc])
        ka = const.tile([128, NP_TILES * OUTROWS], g2dt)
        for hi in range(4):
            hc = slice(hi * 8 * OUTROWS, (hi + 1) * 8 * OUTROWS)
            nc.gpsimd.dma_start(ka[:, hc], ka_d[:, hc])

        spool = ctx.enter_context(tc.tile_pool(name="spool", bufs=3, space="PSUM"))
        opool = ctx.enter_context(tc.tile_pool(name="opool", bufs=2, space="PSUM"))
        epool = ctx.enter_context(tc.tile_pool(name="epool", bufs=4))
        obuf = ctx.enter_context(tc.tile_pool(name="obuf", bufs=2))

        stages = [(ci, pi) for ci in range(NCHUNK) for pi in range(NPAIRS)]
        s_tiles = [None] * len(stages)

        def emit_gemm1(k):
            ci, pi = stages[k]
            s = spool.tile([128, 2 * CHUNK], f32, tag="s")
            s_tiles[k] = s
            pcols = slice(pi * 128, (pi + 1) * 128)
            ccols = slice(ci * CHUNK, (ci + 1) * CHUNK)
            nc.tensor.matmul(s[:, 0:CHUNK], kt[0:64, pcols], rhs[0:64, ccols],
                             start=True, stop=True, tile_position=(0, 0))
            nc.tensor.matmul(s[:, CHUNK:2 * CHUNK], kt[64:128, pcols],
                             rhs[64:128, ccols],
                             start=True, stop=True, tile_position=(64, 0))

        osum = None
        emit_gemm1(0)
        for k, (ci, pi) in enumerate(stages):
            if k + 1 < len(stages):
                emit_gemm1(k + 1)
            if pi == 0:
                osum = opool.tile([128, CHUNK], f32, tag="osum")
            s = s_tiles[k]
            e = epool.tile([128, 2 * CHUNK], g2dt, tag="e")
            nc.scalar.activation(e[:], s[:], mybir.ActivationFunctionType.Exp)
            # p-tile 2*pi (kt2 rows 0:64 -> e slot 0), 2*pi+1 (slot 1)
            for j in range(2):
                t = 2 * pi + j
                for q in range(4):
                    nc.tensor.matmul(
                        osum[32 * q:32 * q + 17, :],
                        ka[:, t * OUTROWS + 17 * q: t * OUTROWS + 17 * (q + 1)],
                        e[:, j * CHUNK:(j + 1) * CHUNK],
                        start=(t == 0), stop=(t == NP_TILES - 1),
                        tile_position=(0, 32 * q),
                        skip_group_check=True,
                    )
            s_tiles[k] = None
            if pi == NPAIRS - 1:
                ob = obuf.tile([128, CHUNK], f32, tag="ob")
                nc.vector.tensor_copy(ob[:], osum[:])
                nc.sync.dma_start(out_d[:, ci * CHUNK:(ci + 1) * CHUNK], ob[:])
    nc.compile()
    return nc


def _get_program():
    if "nc" not in _CACHE:
        _CACHE["nc"] = _build_program()
    return _CACHE["nc"]


def _pool3x3(x):
    # 3x3 stride-1 zero-padded sum pool over the last two axes.
    p = np.pad(x, ((0, 0), (0, 0), (1, 1), (0, 0)))
    x = p[:, :, :-2] + p[:, :, 1:-1] + p[:, :, 2:]
    p = np.pad(x, ((0, 0), (0, 0), (0, 0), (1, 1)))
    return p[:, :, :, :-2] + p[:, :, :, 1:-1] + p[:, :, :, 2:]


def kernel(foreground, masks=None, **_unused):
    global LAST_RESULTS
    from concourse import bass_utils
    import ml_dtypes

    _np_dt = {"bfloat16": ml_dtypes.bfloat16, "float16": np.float16,
              "float32r": np.float32}
    g1np, g2np = _np_dt[G1DT], _np_dt[G2DT]

    fg = np.ascontiguousarray(np.asarray(foreground, dtype=np.float32))
    assert fg.shape == (B, C, H, W)

    # kern_t[c, p] = normalized (fg + eps), kern transposed
    kt_all = fg.reshape(B, C, P) + EPS
    kt_all = kt_all / np.sqrt(
        (kt_all.astype(np.float64) ** 2).sum(1, keepdims=True)).astype(np.float32)
    # kt2: [128, NPAIRS*128] — even p-tiles in rows 0:64, odd in rows 64:128
    kt_r = kt_all.reshape(B, C, NPAIRS, 2, 128)
    kt2 = np.concatenate([kt_r[:, :, :, 0, :].reshape(B, C, NPAIRS * 128),
                          kt_r[:, :, :, 1, :].reshape(B, C, NPAIRS * 128)],
                         axis=1).astype(g1np)
    # ka2: [128, NP_TILES*68] — per p-tile, 4 quarters of (16 kern cols + ones)
    kq = kt_all.transpose(0, 2, 1).reshape(B, NP_TILES, 128, 4, 16)
    kq = np.concatenate([kq, np.ones((B, NP_TILES, 128, 4, 1), np.float32)], -1)
    ka2 = np.ascontiguousarray(kq.transpose(0, 2, 1, 3, 4)).reshape(
        B, 128, NP_TILES * OUTROWS).astype(g2np)

    fg2 = _pool3x3(fg)

    in_maps = []
    for core in range(8):
        b, yh = core // 2, core % 2
        half = fg2[b, :, yh * (H // 2):(yh + 1) * (H // 2), :].reshape(C, YXH)
        in_maps.append({
            "kt2": np.ascontiguousarray(kt2[b]),
            "ka2": np.ascontiguousarray(ka2[b]),
            "rhs2": np.concatenate([half, half], axis=0).astype(g1np),
        })

    nc = _get_program()
    res = bass_utils.run_bass_kernel_spmd(
        nc, in_maps, core_ids=list(range(8)), trace=TRACE)
    LAST_RESULTS = res

    out = np.empty((B, C, H, W), dtype=np.float32)
    for core in range(8):
        b, yh = core // 2, core % 2
        oa = res.results[core]["out_aug"]  # [128, YXH]
        den = oa[16]                       # ones-row of quarter 0
        num = np.concatenate([oa[32 * q:32 * q + 16] for q in range(4)], axis=0)
        img = num / den
        out[b, :, yh * (H // 2):(yh + 1) * (H // 2), :] = img.reshape(C, H // 2, W)
    return out
